# revision 67
# baseline (speedup 1.0000x reference)
"""Trainium2 Bass kernel for nn_AnnsHNSW (retrieval kNN + anns pairing), v3.

Full inputs: query [2,16,2048,64] f32, key [2,16,2048,64] f32, sample_size=64.
Output: (query_sort_idx [2,16,2048] i32, key_pick_idx [2,16,2048] i32).

Math note: the reference's QNF augmentation adds |k_aug|^2 == kmax^2 (a
constant) to every key, and scales each query by r_q > 0.  Both are
order-preserving per query, so the kNN ordering equals ordering by the plain
inner product q.k (descending).

v3 architecture (vs v2's Max8/MaxIndex scans):
- q/k are transposed on the host to [d, n] so the PE consumes them directly
  (no on-chip transposes, no ACT copies for preproc).
- labels: per q-tile, fp32 scores [128,2048] in PSUM -> DVE prefix-max scan
  (tensor_tensor_scan, one pass) -> label = #{P < P[-1]} counted in ONE pass
  on DVE (tensor_scalar is_lt + reduce-add accum, 2x SBUF mode) or ACT
  (Sign + accum).  Exact fp32 compare; first-occurrence ties for free.
- rank: count-less-than on the combined key c = label*2048 + qidx, one
  counting pass per query column, split between DVE and ACT to balance.
- qsi via per-column indirect scatters (Pool/SWDGE).
- kpi: picked one-hot extraction + fp32 picked scores quantized on the
  PSUM->SBUF move to round(v*2^13)+2^19 (int32), packed with the reversed
  column index ((i<<11)|rev, unique positive int32).  The f32 BITCAST of a
  positive int32 is order-isomorphic, so the top-64 needs only 8 rounds of
  Max8+MatchReplace (no MaxIndex); idx = (p & 2047) ^ 2047.  Quantization
  at 2^-13 costs ~33 kpi tie artifacts (relerr 1.03e-2 < 2e-2 gate).
"""

import os

import numpy as np

B, H, NQ, NK, D = 2, 16, 2048, 2048, 64
SAMPLE = 64
N_CORES = 8
SL = (B * H) // N_CORES  # slices per core

NEG_BIG = -1.0e30

# per-(slice,tile) engine assignment for label counts and rank columns:
# True -> DVE, False -> ACT.  ~6 of 32 units per slice on DVE (DVE also
# carries the scans and the top-64 tail).
CNT_DVE = [False] * 16
RANK_DVE = [True, False] * 8
RANK_DVE_TAIL = [(t % 3 != 0) for t in range(16)]


def build_bass(n_slices=SL, nq=NQ, nk=NK, d=D, sample=SAMPLE, split_waits=True,
               sim_safe=False, debug=False):
    import concourse.bass as bass
    import concourse.mybir as mybir
    from concourse.tile import TileContext
    from concourse.masks import make_identity

    f32 = mybir.dt.float32
    i32 = mybir.dt.int32
    u32 = mybir.dt.uint32
    AF = mybir.ActivationFunctionType
    ALU = mybir.AluOpType

    nqt = nq // 128          # q tiles per slice (16)
    npick = nq // sample     # picked queries per slice (32)
    nused = npick * n_slices
    assert nused <= 128

    nc = bass.Bass()
    qt_in = nc.declare_dram_parameter("qt", [n_slices, d, nq], f32, isOutput=False)
    kt_in = nc.declare_dram_parameter("kt", [n_slices, d, nk], f32, isOutput=False)
    qrow_in = nc.declare_dram_parameter("qrow", [n_slices * nq, d], f32, isOutput=False)
    qiota_f_in = nc.declare_dram_parameter("qiota_f", [128, nqt], f32, isOutput=False)
    qiota_i_in = nc.declare_dram_parameter("qiota_i", [128, nqt], i32, isOutput=False)
    c64_in = nc.declare_dram_parameter("c64", [128, npick], f32, isOutput=False)
    revi_in = nc.declare_dram_parameter("revi", [128, nk], i32, isOutput=False)
    qsi_out = nc.declare_dram_parameter("qsi", [n_slices * nq, 1], i32, isOutput=True)
    kpi_out = nc.declare_dram_parameter("kpi", [n_slices, nq], i32, isOutput=True)

    crow_dram = nc.dram_tensor("crow_dram", [n_slices, nq], f32)
    if debug:
        dbg_lab = nc.declare_dram_parameter("dbg_lab", [n_slices, 128, nqt], f32, isOutput=True)
        dbg_rank = nc.declare_dram_parameter("dbg_rank", [n_slices, 128, nqt], f32, isOutput=True)
        dbg_pick = nc.declare_dram_parameter("dbg_pick", [n_slices, npick], i32, isOutput=True)

    with TileContext(nc) as tc:
        with (
            tc.tile_pool(name="const", bufs=1) as constp,
            tc.tile_pool(name="ktp", bufs=3) as ktp,
            tc.tile_pool(name="qtp", bufs=2) as qtp,
            tc.tile_pool(name="Pp", bufs=4) as Pp,
            tc.tile_pool(name="scrD", bufs=1) as scrDp,
            tc.tile_pool(name="scrA", bufs=1) as scrAp,
            tc.tile_pool(name="cbp", bufs=2) as cbp,
            tc.tile_pool(name="smallp", bufs=2) as smallp,
            tc.tile_pool(name="pickp", bufs=2) as pickp,
            tc.tile_pool(name="finalp", bufs=1) as finalp,
            tc.tile_pool(name="v8p", bufs=4) as v8p,
            tc.tile_pool(name="ps_scA", bufs=1, space="PSUM") as ps_scAp,
            tc.tile_pool(name="ps_scB", bufs=1, space="PSUM") as ps_scBp,
            tc.tile_pool(name="ps_tr", bufs=2, space="PSUM") as ps_trp,
            tc.tile_pool(name="ps_pp", bufs=2, space="PSUM") as ps_ppp,
        ):
            # ---- constants (DMAs issued on the ACT queue AFTER the first
            # slice loads, so the sync queue's serial 650ns/DMA slots go to
            # the critical kt/qt chunks first; revi is tail-only anyway) ----
            ident = constp.tile([128, 128], f32, tag="ident")
            make_identity(nc, ident[:])
            qiota_f = constp.tile([128, nqt], f32, tag="qiota_f")
            qiota_i = constp.tile([128, nqt], i32, tag="qiota_i")
            c64 = constp.tile([128, npick], f32, tag="c64")
            revi = constp.tile([128, nk], i32, tag="revi")
            sh11 = constp.tile([128, 1], i32, tag="sh11")
            nc.vector.memset(sh11[:], 11)
            negbig = constp.tile([128, 1], f32, tag="negbig")
            nc.vector.memset(negbig[:], NEG_BIG)

            def load_consts():
                nc.scalar.dma_start(qiota_f[:], qiota_f_in[:])
                nc.scalar.dma_start(qiota_i[:], qiota_i_in[:])
                nc.scalar.dma_start(c64[:], c64_in[:])
                nc.scalar.dma_start(revi[:], revi_in[:])

            # persistent PSUM, even split: PE refills scA while scB scans
            NA = nk // 2
            scA = ps_scAp.tile([128, NA], f32, tag="scA")
            scB = ps_scBp.tile([128, nk - NA], f32, tag="scB")

            # persistent SBUF
            scrD = scrDp.tile([128, nk], f32, tag="scrD")
            scrA = scrAp.tile([128, nk], f32, tag="scrA")
            pqt = finalp.tile([d, nused], f32, tag="pqt")
            # psc holds round(v*2^13)+2^19 as int32; packed with the reversed
            # column index it stays a positive int32, whose f32 BITCAST is
            # order-isomorphic -> Max8/MatchReplace rounds need no MaxIndex
            psc = finalp.tile([nused, nk], i32, tag="psc")
            psc_p = finalp.tile([nused, nk], i32, tag="psc_p")
            topP = finalp.tile([nused, sample], f32, tag="topP")
            topidx = finalp.tile([nused, sample], i32, tag="topidx")

            # warmups: dummy PE matmul (absorbs ident's gpsimd sem) and an
            # ACT Sign op (loads the act table before the critical path)
            wtr = ps_trp.tile([128, 128], f32, tag="ps_tr")
            nc.tensor.matmul(wtr[:], lhsT=ident[:], rhs=ident[:], start=True, stop=True)
            dscrap = constp.tile([1, 1], f32, tag="dscrap")
            nc.vector.tensor_copy(dscrap[:], wtr[0:1, 0:1])
            wsig = constp.tile([1, 1], f32, tag="wsig")
            nc.scalar.activation(wsig[:], negbig[0:1, 0:1], AF.Sign, bias=0.0, scale=1.0)

            kts = {}
            qts = {}
            cts = {}
            accs = {}
            labs = {}
            cs = {}
            cbs = {}
            ranks = {}
            rankis = {}
            pickis = {}
            pqs = {}

            def load_kq(s):
                if s >= n_slices:
                    return
                # chunked loads so tile-0 matmuls start before the full
                # 512KB transfer lands
                kt = ktp.tile([d, nk], f32, tag="kt", name="kt")
                qt = qtp.tile([d, nq], f32, tag="qt", name="qt")
                for ch in range(4):
                    nc.gpsimd.dma_start(qt[:, ch * 512:(ch + 1) * 512],
                                        qt_in[s, :, ch * 512:(ch + 1) * 512])
                    nc.sync.dma_start(kt[:, ch * 512:(ch + 1) * 512],
                                      kt_in[s, :, ch * 512:(ch + 1) * 512])
                kts[s], qts[s] = kt, qt

            def alloc_lab(s):
                labs[s] = smallp.tile([128, nqt], f32, tag="lab", name="lab")

            def scores(s, t):
                kt, qt = kts[s], qts[s]
                lhs = qt[:, t * 128:(t + 1) * 128]
                for ch in range(2):
                    nc.tensor.matmul(scA[:, ch * 512:(ch + 1) * 512], lhsT=lhs,
                                     rhs=kt[:, ch * 512:(ch + 1) * 512],
                                     start=True, stop=True)
                for ch in range(2):
                    nc.tensor.matmul(scB[:, ch * 512:(ch + 1) * 512], lhsT=lhs,
                                     rhs=kt[:, NA + ch * 512:NA + (ch + 1) * 512],
                                     start=True, stop=True)

            def scan_count(s, t):
                """Chained prefix-max scan of scA/scB + one-pass label count.
                Two scan parts so the PE can refill scA while scB scans."""
                P = Pp.tile([128, nk], f32, tag="P", name="P")
                nc.vector.tensor_tensor_scan(
                    P[:, 0:NA], scA[:], negbig[:].broadcast_to([128, NA]),
                    initial=NEG_BIG, op0=ALU.max, op1=ALU.max)
                nc.vector.tensor_tensor_scan(
                    P[:, NA:nk], scB[:], negbig[:].broadcast_to([128, nk - NA]),
                    initial=P[:, NA - 1:NA], op0=ALU.max, op1=ALU.max)
                lab = labs[s]
                if CNT_DVE[t] or (s == n_slices - 1 and t >= 12):
                    nc.vector.tensor_scalar(scrD[:], P[:], P[:, nk - 1:nk], None,
                                            op0=ALU.is_lt, op1=ALU.add,
                                            accum_out=lab[:, t:t + 1])
                else:
                    nc.scalar.activation(scrA[:], P[:], AF.Sign, bias=P[:, nk - 1:nk],
                                         scale=-1.0, accum_out=lab[:, t:t + 1])

            def merge(s):
                """c = lab*2048 + qidx."""
                lab = labs[s]
                c = smallp.tile([128, nqt], f32, tag="c", name="c")
                nc.vector.tensor_scalar(c[:], lab[:], float(nq), None, op0=ALU.mult)
                nc.vector.tensor_tensor(c[:], c[:], qiota_f[:], op=ALU.add)
                cs[s] = c
                if debug:
                    nc.sync.dma_start(dbg_lab[s], lab[:])
                ranks[s] = smallp.tile([128, nqt], f32, tag="rank", name="rank")
                rankis[s] = smallp.tile([128, nqt], i32, tag="ranki", name="ranki")
                accs[s] = smallp.tile([128, nqt], f32, tag="acc", name="acc")

            def rank_tr(s, store=True):
                """c [128,16] -> ct [16,128] (PE transpose), optionally stored
                to crow_dram for the DMA-broadcast cb path."""
                c = cs[s]
                ptr = ps_trp.tile([128, 128], f32, tag="ps_tr")
                nc.tensor.transpose(ptr[0:nqt, :], c[:], ident[:])
                ct = smallp.tile([nqt, 128], f32, tag="ct")
                nc.scalar.copy(ct[:], ptr[0:nqt, :])
                cts[s] = ct
                if store:
                    nc.scalar.dma_start(
                        crow_dram[s].rearrange("(t p) -> t p", t=nqt), ct[:]
                    )


            def cb_build(s):
                cb = cbp.tile([128, nq], f32, tag="cb")
                nc.scalar.dma_start(cb[:], crow_dram[s:s + 1, :].broadcast_to([128, nq]))
                cbs[s] = cb

            def rank_col(s, t, tail=False):
                """rank[:, t] = #{c' < c[:, t]} in one counting pass.
                ACT's Sign gives #less - #greater, fixed up by (x+2047)/2."""
                c, cb, rank = cs[s], cbs[s], ranks[s]
                if (RANK_DVE_TAIL if tail else RANK_DVE)[t]:
                    nc.vector.tensor_scalar(scrD[:], cb[:], c[:, t:t + 1], None,
                                            op0=ALU.is_lt, op1=ALU.add,
                                            accum_out=rank[:, t:t + 1])
                else:
                    acc = accs[s]
                    nc.scalar.activation(scrA[:], cb[:], AF.Sign, bias=c[:, t:t + 1],
                                         scale=-1.0, accum_out=acc[:, t:t + 1])
                    nc.vector.tensor_scalar(rank[:, t:t + 1], acc[:, t:t + 1],
                                            float(nq - 1), 0.5,
                                            op0=ALU.add, op1=ALU.mult)
                nc.vector.tensor_copy(rankis[s][:, t:t + 1], rank[:, t:t + 1])

            def picked_extract_col(s, t, ps_pk):
                """One-hot A_t = [rank_t == 64p'] and accumulate A_t^T qiota_t."""
                rank = ranks[s]
                A = smallp.tile([128, npick], f32, tag="Aoh")
                nc.vector.tensor_tensor(
                    A[:], rank[:, t:t + 1].broadcast_to([128, npick]), c64[:],
                    op=ALU.is_equal)
                nc.tensor.matmul(ps_pk[:], lhsT=A[:], rhs=qiota_f[:, t:t + 1],
                                 start=(t == 0), stop=(t == nqt - 1))

            def picked_fin(s, ps_pk):
                picki = pickp.tile([npick, 1], i32, tag="picki")
                nc.vector.tensor_copy(picki[:], ps_pk[:])
                pickis[s] = picki
                if debug:
                    nc.sync.dma_start(dbg_rank[s], ranks[s][:])
                    nc.sync.dma_start(dbg_pick[s].rearrange("j -> j ()"), picki[:])

            def picked_extract(s):
                ps_pk = ps_trp.tile([npick, 1], f32, tag="ps_tr")
                for t in range(nqt):
                    picked_extract_col(s, t, ps_pk)
                picked_fin(s, ps_pk)

            def picked_gather(s):
                pq = pickp.tile([npick, d], f32, tag="pq")
                pqs[s] = pq
                grows = n_slices * nq if sim_safe else npick
                nc.gpsimd.indirect_dma_start(
                    out=pq[:],
                    out_offset=None,
                    in_=qrow_in[0:grows, :],
                    in_offset=bass.IndirectOffsetOnAxis(ap=pickis[s][:], axis=0),
                    element_offset=s * nq * d,
                )

            def scatters(s, t0=0, cnt=nqt):
                ranki = rankis[s]
                nrows = n_slices * nq if sim_safe else 128
                for t in range(t0, t0 + cnt):
                    # out AP window kept small: SWDGE descriptor count (and the
                    # cost model) size by the declared AP, not the 128 writes
                    nc.gpsimd.indirect_dma_start(
                        out=qsi_out[0:nrows, :],
                        out_offset=bass.IndirectOffsetOnAxis(
                            ap=ranki[:, t:t + 1], axis=0),
                        in_=qiota_i[:, t:t + 1],
                        in_offset=None,
                        element_offset=s * nq,
                    )

            def picked_scores(s):
                pq = pqs[s]
                ptr = ps_trp.tile([128, 128], f32, tag="ps_tr")
                nc.tensor.transpose(ptr[0:d, 0:npick], pq[:], ident[0:npick, 0:npick])
                nc.scalar.copy(pqt[0:d, s * npick:(s + 1) * npick], ptr[0:d, 0:npick])
                last = s == n_slices - 1
                for n in range(nk // 256):
                    ps_pp = ps_ppp.tile([nused, 256], f32, tag="ps_pp")
                    nc.tensor.matmul(
                        ps_pp[s * npick:(s + 1) * npick, :],
                        lhsT=pqt[:, s * npick:(s + 1) * npick],
                        rhs=kts[s][:, n * 256:(n + 1) * 256],
                        start=True, stop=True,
                        tile_position=(0, s * npick),
                    )
                    dst = psc[s * npick:(s + 1) * npick, n * 256:(n + 1) * 256]
                    if last:
                        # DVE copy: the ACT queue is busy with the tail ladder
                        nc.vector.tensor_scalar(dst, ps_pp[s * npick:(s + 1) * npick, :],
                                                8192.0, float(1 << 19),
                                                op0=ALU.mult, op1=ALU.add)
                    else:
                        nc.scalar.activation(dst, ps_pp[s * npick:(s + 1) * npick, :],
                                             AF.Copy, bias=float(1 << 19), scale=8192.0)

            # ================= startup =================
            load_kq(0)
            load_consts()

            # ================= main pipeline =================
            for s in range(n_slices):
                alloc_lab(s)
                for t in range(nqt):
                    scores(s, t)
                    scan_count(s, t)
                    # hooks for slice s-1 work AFTER the scans in program
                    # order so they never delay the PE's scA/scB refill
                    if t == 0:
                        load_kq(s + 1)
                        if s >= 1:
                            merge(s - 1)
                    elif t == 1 and s >= 1:
                        rank_tr(s - 1)
                    elif t == 2 and s >= 1:
                        cb_build(s - 1)
                    elif t == 13 and s >= 1:
                        picked_extract(s - 1)
                    elif t == 14 and s >= 1:
                        picked_gather(s - 1)   # Pool: before the scatters
                        scatters(s - 1)
                    elif t == 15 and s >= 1:
                        picked_scores(s - 1)
                    if 3 <= t <= 10 and s >= 1:
                        rank_col(s - 1, (t - 3) * 2)
                        rank_col(s - 1, (t - 3) * 2 + 1)

            # ================= tail: slice n-1 rank + picked =================
            sl = n_slices - 1
            merge(sl)
            rank_tr(sl)
            cb_build(sl)
            ps_pk = ps_trp.tile([npick, 1], f32, tag="ps_tr")
            for t in range(nqt):
                rank_col(sl, t, tail=True)
                picked_extract_col(sl, t, ps_pk)
            picked_fin(sl, ps_pk)
            picked_gather(sl)      # Pool: ahead of the final scatters
            scatters(sl)           # drain concurrently with extraction
            picked_scores(sl)

            # ================= top-64 extraction =================
            # pack: psc_p = psc*2048 + (2047 - col); unique positive int32
            nc.vector.scalar_tensor_tensor(
                psc_p[:], psc[:], sh11[0:nused, 0:1], revi[0:nused, :],
                op0=ALU.logical_shift_left, op1=ALU.bitwise_or)
            psc_f = psc_p[:].bitcast(f32)
            # per-round idx extraction + kpi stores on non-DVE queues, so the
            # output path hides under the rounds instead of trailing them
            store_qs = [nc.sync, nc.scalar, nc.gpsimd, nc.sync]
            for r in range(sample // 8):
                pv8 = v8p.tile([nused, 8], f32, tag="pv8")
                nc.vector.max(out=pv8[:], in_=psc_f)
                nc.vector.tensor_copy(topP[:, r * 8:(r + 1) * 8], pv8[:])
                # idx = 2047 - (packed & 2047) == (packed & 2047) ^ 2047
                nc.vector.tensor_scalar(
                    topidx[:, r * 8:(r + 1) * 8],
                    topP[:, r * 8:(r + 1) * 8].bitcast(i32), 2047, 2047,
                    op0=ALU.bitwise_and, op1=ALU.bitwise_xor)
                for s in range(n_slices):
                    store_qs[s].dma_start(
                        kpi_out[s].rearrange("(j k) -> j k", k=sample)[:, r * 8:(r + 1) * 8],
                        topidx[s * npick:(s + 1) * npick, r * 8:(r + 1) * 8],
                    )
                if r < sample // 8 - 1:
                    nc.vector.match_replace(
                        out=psc_f, in_to_replace=pv8[:], in_values=psc_f,
                        imm_value=NEG_BIG,
                    )

    if split_waits:
        import concourse.mybir as mybir_mod
        _split_multi_waits(nc, mybir_mod)
    return nc


def _split_multi_waits(nc, mybir):
    """Walrus accepts only ONE sync-wait per instruction; move extras onto
    same-engine NoOps inserted before the offending instruction."""
    n = 0
    for f in nc.m.functions:
        for blk in f.blocks:
            out = []
            for inst in blk.instructions:
                si = getattr(inst, "sync_info", None)
                if si is not None and len(si.on_wait) > 1:
                    waits = list(si.on_wait)
                    for w in waits[:-1]:
                        nop = mybir.InstNoOp(
                            name=f"I-wsplit-{n}", ins=[], outs=[],
                            text_hint="wsplit",
                        )
                        n += 1
                        nop.engine = inst.engine
                        nop.sync_info = mybir.SyncInfo(on_wait=[w], on_update=[])
                        out.append(nop)
                    inst.sync_info = mybir.SyncInfo(
                        on_wait=[waits[-1]], on_update=list(si.on_update)
                    )
                out.append(inst)
            blk.instructions = out
    return nc


_BUILT = {}
LAST_RESULTS = None


def _get_nc(key=(SL, NQ, NK, D, SAMPLE)):
    if key not in _BUILT:
        _BUILT[key] = build_bass(*key)
    return _BUILT[key]


def make_iota(nqt=NQ // 128):
    # qiota[p, t] = t*128 + p
    p = np.arange(128, dtype=np.int64)[:, None]
    t = np.arange(nqt, dtype=np.int64)[None, :]
    v = (t * 128 + p)
    return v.astype(np.float32), v.astype(np.int32)


def kernel(query, key, sample_size=SAMPLE):
    from concourse.bass_utils import run_bass_kernel_spmd

    q = np.ascontiguousarray(np.asarray(query, dtype=np.float32)).reshape(B * H, NQ, D)
    k = np.ascontiguousarray(np.asarray(key, dtype=np.float32)).reshape(B * H, NK, D)
    iota_f, iota_i = make_iota()
    c64 = np.broadcast_to(
        (np.arange(NQ // SAMPLE, dtype=np.float32) * SAMPLE)[None, :], (128, NQ // SAMPLE)
    ).copy()
    revi = np.ascontiguousarray(
        np.broadcast_to((2047 - np.arange(NK, dtype=np.int32))[None, :], (128, NK))
    )

    in_maps = []
    for c in range(N_CORES):
        qs = q[c * SL:(c + 1) * SL]                       # [SL, NQ, D]
        ks = k[c * SL:(c + 1) * SL]
        in_maps.append(
            {
                "qt": np.ascontiguousarray(qs.transpose(0, 2, 1)),   # [SL, D, NQ]
                "kt": np.ascontiguousarray(ks.transpose(0, 2, 1)),
                "qrow": np.ascontiguousarray(qs.reshape(SL * NQ, D)),
                "qiota_f": iota_f,
                "qiota_i": iota_i,
                "c64": c64,
                "revi": revi,
            }
        )

    nc = _get_nc()
    trace = bool(os.environ.get("ANNS_TRACE"))
    res = run_bass_kernel_spmd(
        nc, in_maps, core_ids=list(range(N_CORES)), trace=trace
    )
    global LAST_RESULTS
    LAST_RESULTS = res
    qsi = np.concatenate(
        [res.results[i]["qsi"].reshape(SL, NQ) for i in range(N_CORES)], axis=0
    ).reshape(B, H, NQ)
    kpi = np.concatenate(
        [res.results[i]["kpi"].reshape(SL, NQ) for i in range(N_CORES)], axis=0
    ).reshape(B, H, NQ)
    return qsi.astype(np.int32), kpi.astype(np.int32)


if __name__ == "__main__":
    rng = np.random.default_rng(0)
    q = rng.normal(size=(B, H, NQ, D)).astype(np.float32)
    k = rng.normal(size=(B, H, NK, D)).astype(np.float32)
    out = kernel(q, k, SAMPLE)
    print([o.shape for o in out])


# revision 68
# speedup vs baseline: 1.0009x; 1.0009x over previous
"""Trainium2 Bass kernel for nn_AnnsHNSW (retrieval kNN + anns pairing), v3.

Full inputs: query [2,16,2048,64] f32, key [2,16,2048,64] f32, sample_size=64.
Output: (query_sort_idx [2,16,2048] i32, key_pick_idx [2,16,2048] i32).

Math note: the reference's QNF augmentation adds |k_aug|^2 == kmax^2 (a
constant) to every key, and scales each query by r_q > 0.  Both are
order-preserving per query, so the kNN ordering equals ordering by the plain
inner product q.k (descending).

v3 architecture (vs v2's Max8/MaxIndex scans):
- q/k are transposed on the host to [d, n] so the PE consumes them directly
  (no on-chip transposes, no ACT copies for preproc).
- labels: per q-tile, fp32 scores [128,2048] in PSUM -> DVE prefix-max scan
  (tensor_tensor_scan, one pass) -> label = #{P < P[-1]} counted in ONE pass
  on DVE (tensor_scalar is_lt + reduce-add accum, 2x SBUF mode) or ACT
  (Sign + accum).  Exact fp32 compare; first-occurrence ties for free.
- rank: count-less-than on the combined key c = label*2048 + qidx, one
  counting pass per query column, split between DVE and ACT to balance.
- qsi via per-column indirect scatters (Pool/SWDGE).
- kpi: picked one-hot extraction + fp32 picked scores quantized on the
  PSUM->SBUF move to round(v*2^13)+2^19 (int32), packed with the reversed
  column index ((i<<11)|rev, unique positive int32).  The f32 BITCAST of a
  positive int32 is order-isomorphic, so the top-64 needs only 8 rounds of
  Max8+MatchReplace (no MaxIndex); idx = (p & 2047) ^ 2047.  Quantization
  at 2^-13 costs ~33 kpi tie artifacts (relerr 1.03e-2 < 2e-2 gate).
"""

import os

import numpy as np

B, H, NQ, NK, D = 2, 16, 2048, 2048, 64
SAMPLE = 64
N_CORES = 8
SL = (B * H) // N_CORES  # slices per core

NEG_BIG = -1.0e30

# per-(slice,tile) engine assignment for label counts and rank columns:
# True -> DVE, False -> ACT.  ~6 of 32 units per slice on DVE (DVE also
# carries the scans and the top-64 tail).
CNT_DVE = [False] * 16
RANK_DVE = [True, False] * 8
RANK_DVE_TAIL = [(t % 3 != 0) for t in range(16)]


def build_bass(n_slices=SL, nq=NQ, nk=NK, d=D, sample=SAMPLE, split_waits=True,
               sim_safe=False, debug=False):
    import concourse.bass as bass
    import concourse.mybir as mybir
    from concourse.tile import TileContext
    from concourse.masks import make_identity

    f32 = mybir.dt.float32
    i32 = mybir.dt.int32
    u32 = mybir.dt.uint32
    AF = mybir.ActivationFunctionType
    ALU = mybir.AluOpType

    nqt = nq // 128          # q tiles per slice (16)
    npick = nq // sample     # picked queries per slice (32)
    nused = npick * n_slices
    assert nused <= 128

    nc = bass.Bass()
    qt_in = nc.declare_dram_parameter("qt", [n_slices, d, nq], f32, isOutput=False)
    kt_in = nc.declare_dram_parameter("kt", [n_slices, d, nk], f32, isOutput=False)
    qrow_in = nc.declare_dram_parameter("qrow", [n_slices * nq, d], f32, isOutput=False)
    qiota_f_in = nc.declare_dram_parameter("qiota_f", [128, nqt], f32, isOutput=False)
    qiota_i_in = nc.declare_dram_parameter("qiota_i", [128, nqt], i32, isOutput=False)
    c64_in = nc.declare_dram_parameter("c64", [128, npick], f32, isOutput=False)
    revi_in = nc.declare_dram_parameter("revi", [128, nk], i32, isOutput=False)
    qsi_out = nc.declare_dram_parameter("qsi", [n_slices * nq, 1], i32, isOutput=True)
    kpi_out = nc.declare_dram_parameter("kpi", [n_slices, nq], i32, isOutput=True)

    crow_dram = nc.dram_tensor("crow_dram", [n_slices, nq], f32)
    if debug:
        dbg_lab = nc.declare_dram_parameter("dbg_lab", [n_slices, 128, nqt], f32, isOutput=True)
        dbg_rank = nc.declare_dram_parameter("dbg_rank", [n_slices, 128, nqt], f32, isOutput=True)
        dbg_pick = nc.declare_dram_parameter("dbg_pick", [n_slices, npick], i32, isOutput=True)

    with TileContext(nc) as tc:
        with (
            tc.tile_pool(name="const", bufs=1) as constp,
            tc.tile_pool(name="ktp", bufs=3) as ktp,
            tc.tile_pool(name="qtp", bufs=2) as qtp,
            tc.tile_pool(name="Pp", bufs=4) as Pp,
            tc.tile_pool(name="scrD", bufs=1) as scrDp,
            tc.tile_pool(name="scrA", bufs=1) as scrAp,
            tc.tile_pool(name="cbp", bufs=2) as cbp,
            tc.tile_pool(name="smallp", bufs=2) as smallp,
            tc.tile_pool(name="pickp", bufs=2) as pickp,
            tc.tile_pool(name="finalp", bufs=1) as finalp,
            tc.tile_pool(name="v8p", bufs=4) as v8p,
            tc.tile_pool(name="ps_scA", bufs=1, space="PSUM") as ps_scAp,
            tc.tile_pool(name="ps_scB", bufs=1, space="PSUM") as ps_scBp,
            tc.tile_pool(name="ps_tr", bufs=2, space="PSUM") as ps_trp,
            tc.tile_pool(name="ps_pp", bufs=2, space="PSUM") as ps_ppp,
        ):
            # ---- constants (DMAs issued on the ACT queue AFTER the first
            # slice loads, so the sync queue's serial 650ns/DMA slots go to
            # the critical kt/qt chunks first; revi is tail-only anyway) ----
            ident = constp.tile([128, 128], f32, tag="ident")
            make_identity(nc, ident[:])
            qiota_f = constp.tile([128, nqt], f32, tag="qiota_f")
            qiota_i = constp.tile([128, nqt], i32, tag="qiota_i")
            c64 = constp.tile([128, npick], f32, tag="c64")
            revi = constp.tile([128, nk], i32, tag="revi")
            sh11 = constp.tile([128, 1], i32, tag="sh11")
            nc.vector.memset(sh11[:], 11)
            negbig = constp.tile([128, 1], f32, tag="negbig")
            nc.vector.memset(negbig[:], NEG_BIG)

            def load_consts():
                nc.scalar.dma_start(qiota_f[:], qiota_f_in[:])
                nc.scalar.dma_start(qiota_i[:], qiota_i_in[:])
                nc.scalar.dma_start(c64[:], c64_in[:])
                nc.scalar.dma_start(revi[:], revi_in[:])

            # persistent PSUM, even split: PE refills scA while scB scans
            NA = nk // 2
            scA = ps_scAp.tile([128, NA], f32, tag="scA")
            scB = ps_scBp.tile([128, nk - NA], f32, tag="scB")

            # persistent SBUF
            scrD = scrDp.tile([128, nk], f32, tag="scrD")
            scrA = scrAp.tile([128, nk], f32, tag="scrA")
            pqt = finalp.tile([d, nused], f32, tag="pqt")
            # psc holds round(v*2^13)+2^19 as int32; packed with the reversed
            # column index it stays a positive int32, whose f32 BITCAST is
            # order-isomorphic -> Max8/MatchReplace rounds need no MaxIndex
            psc = finalp.tile([nused, nk], i32, tag="psc")
            psc_p = finalp.tile([nused, nk], i32, tag="psc_p")
            topP = finalp.tile([nused, sample], f32, tag="topP")
            topidx = finalp.tile([nused, sample], i32, tag="topidx")

            # warmups: dummy PE matmul (absorbs ident's gpsimd sem) and an
            # ACT Sign op (loads the act table before the critical path)
            wtr = ps_trp.tile([128, 128], f32, tag="ps_tr")
            nc.tensor.matmul(wtr[:], lhsT=ident[:], rhs=ident[:], start=True, stop=True)
            dscrap = constp.tile([1, 1], f32, tag="dscrap")
            nc.vector.tensor_copy(dscrap[:], wtr[0:1, 0:1])
            wsig = constp.tile([1, 1], f32, tag="wsig")
            nc.scalar.activation(wsig[:], negbig[0:1, 0:1], AF.Sign, bias=0.0, scale=1.0)

            kts = {}
            qts = {}
            cts = {}
            accs = {}
            labs = {}
            cs = {}
            cbs = {}
            ranks = {}
            rankis = {}
            pickis = {}
            pqs = {}

            def load_kq(s):
                if s >= n_slices:
                    return
                # chunked loads so tile-0 matmuls start before the full
                # 512KB transfer lands
                kt = ktp.tile([d, nk], f32, tag="kt", name="kt")
                qt = qtp.tile([d, nq], f32, tag="qt", name="qt")
                for ch in range(4):
                    nc.gpsimd.dma_start(qt[:, ch * 512:(ch + 1) * 512],
                                        qt_in[s, :, ch * 512:(ch + 1) * 512])
                    nc.sync.dma_start(kt[:, ch * 512:(ch + 1) * 512],
                                      kt_in[s, :, ch * 512:(ch + 1) * 512])
                kts[s], qts[s] = kt, qt

            def alloc_lab(s):
                labs[s] = smallp.tile([128, nqt], f32, tag="lab", name="lab")

            def scores(s, t):
                kt, qt = kts[s], qts[s]
                lhs = qt[:, t * 128:(t + 1) * 128]
                for ch in range(2):
                    nc.tensor.matmul(scA[:, ch * 512:(ch + 1) * 512], lhsT=lhs,
                                     rhs=kt[:, ch * 512:(ch + 1) * 512],
                                     start=True, stop=True)
                for ch in range(2):
                    nc.tensor.matmul(scB[:, ch * 512:(ch + 1) * 512], lhsT=lhs,
                                     rhs=kt[:, NA + ch * 512:NA + (ch + 1) * 512],
                                     start=True, stop=True)

            def scan_count(s, t):
                """Chained prefix-max scan of scA/scB + one-pass label count.
                Two scan parts so the PE can refill scA while scB scans."""
                P = Pp.tile([128, nk], f32, tag="P", name="P")
                nc.vector.tensor_tensor_scan(
                    P[:, 0:NA], scA[:], negbig[:].broadcast_to([128, NA]),
                    initial=NEG_BIG, op0=ALU.max, op1=ALU.max)
                nc.vector.tensor_tensor_scan(
                    P[:, NA:nk], scB[:], negbig[:].broadcast_to([128, nk - NA]),
                    initial=P[:, NA - 1:NA], op0=ALU.max, op1=ALU.max)
                lab = labs[s]
                if CNT_DVE[t] or (s == n_slices - 1 and t >= 12):
                    nc.vector.tensor_scalar(scrD[:], P[:], P[:, nk - 1:nk], None,
                                            op0=ALU.is_lt, op1=ALU.add,
                                            accum_out=lab[:, t:t + 1])
                else:
                    nc.scalar.activation(scrA[:], P[:], AF.Sign, bias=P[:, nk - 1:nk],
                                         scale=-1.0, accum_out=lab[:, t:t + 1])

            def merge(s):
                """c = lab*2048 + qidx."""
                lab = labs[s]
                c = smallp.tile([128, nqt], f32, tag="c", name="c")
                nc.vector.tensor_scalar(c[:], lab[:], float(nq), None, op0=ALU.mult)
                nc.vector.tensor_tensor(c[:], c[:], qiota_f[:], op=ALU.add)
                cs[s] = c
                if debug:
                    nc.sync.dma_start(dbg_lab[s], lab[:])
                ranks[s] = smallp.tile([128, nqt], f32, tag="rank", name="rank")
                rankis[s] = smallp.tile([128, nqt], i32, tag="ranki", name="ranki")
                accs[s] = smallp.tile([128, nqt], f32, tag="acc", name="acc")

            def rank_tr(s, store=True, q=None):
                """c [128,16] -> ct [16,128] (PE transpose), optionally stored
                to crow_dram for the DMA-broadcast cb path."""
                c = cs[s]
                ptr = ps_trp.tile([128, 128], f32, tag="ps_tr")
                nc.tensor.transpose(ptr[0:nqt, :], c[:], ident[:])
                ct = smallp.tile([nqt, 128], f32, tag="ct")
                nc.scalar.copy(ct[:], ptr[0:nqt, :])
                cts[s] = ct
                if store:
                    (q or nc.scalar).dma_start(
                        crow_dram[s].rearrange("(t p) -> t p", t=nqt), ct[:]
                    )


            def cb_build(s, q=None):
                cb = cbp.tile([128, nq], f32, tag="cb")
                (q or nc.scalar).dma_start(
                    cb[:], crow_dram[s:s + 1, :].broadcast_to([128, nq]))
                cbs[s] = cb

            def rank_col(s, t, tail=False):
                """rank[:, t] = #{c' < c[:, t]} in one counting pass.
                ACT's Sign gives #less - #greater, fixed up by (x+2047)/2."""
                c, cb, rank = cs[s], cbs[s], ranks[s]
                if (RANK_DVE_TAIL if tail else RANK_DVE)[t]:
                    nc.vector.tensor_scalar(scrD[:], cb[:], c[:, t:t + 1], None,
                                            op0=ALU.is_lt, op1=ALU.add,
                                            accum_out=rank[:, t:t + 1])
                else:
                    acc = accs[s]
                    nc.scalar.activation(scrA[:], cb[:], AF.Sign, bias=c[:, t:t + 1],
                                         scale=-1.0, accum_out=acc[:, t:t + 1])
                    nc.vector.tensor_scalar(rank[:, t:t + 1], acc[:, t:t + 1],
                                            float(nq - 1), 0.5,
                                            op0=ALU.add, op1=ALU.mult)
                nc.vector.tensor_copy(rankis[s][:, t:t + 1], rank[:, t:t + 1])

            def picked_extract_col(s, t, ps_pk):
                """One-hot A_t = [rank_t == 64p'] and accumulate A_t^T qiota_t."""
                rank = ranks[s]
                A = smallp.tile([128, npick], f32, tag="Aoh")
                nc.vector.tensor_tensor(
                    A[:], rank[:, t:t + 1].broadcast_to([128, npick]), c64[:],
                    op=ALU.is_equal)
                nc.tensor.matmul(ps_pk[:], lhsT=A[:], rhs=qiota_f[:, t:t + 1],
                                 start=(t == 0), stop=(t == nqt - 1))

            def picked_fin(s, ps_pk):
                picki = pickp.tile([npick, 1], i32, tag="picki")
                nc.vector.tensor_copy(picki[:], ps_pk[:])
                pickis[s] = picki
                if debug:
                    nc.sync.dma_start(dbg_rank[s], ranks[s][:])
                    nc.sync.dma_start(dbg_pick[s].rearrange("j -> j ()"), picki[:])

            def picked_extract(s):
                ps_pk = ps_trp.tile([npick, 1], f32, tag="ps_tr")
                for t in range(nqt):
                    picked_extract_col(s, t, ps_pk)
                picked_fin(s, ps_pk)

            def picked_gather(s):
                pq = pickp.tile([npick, d], f32, tag="pq")
                pqs[s] = pq
                grows = n_slices * nq if sim_safe else npick
                nc.gpsimd.indirect_dma_start(
                    out=pq[:],
                    out_offset=None,
                    in_=qrow_in[0:grows, :],
                    in_offset=bass.IndirectOffsetOnAxis(ap=pickis[s][:], axis=0),
                    element_offset=s * nq * d,
                )

            def scatters(s, t0=0, cnt=nqt):
                ranki = rankis[s]
                nrows = n_slices * nq if sim_safe else 128
                for t in range(t0, t0 + cnt):
                    # out AP window kept small: SWDGE descriptor count (and the
                    # cost model) size by the declared AP, not the 128 writes
                    nc.gpsimd.indirect_dma_start(
                        out=qsi_out[0:nrows, :],
                        out_offset=bass.IndirectOffsetOnAxis(
                            ap=ranki[:, t:t + 1], axis=0),
                        in_=qiota_i[:, t:t + 1],
                        in_offset=None,
                        element_offset=s * nq,
                    )

            def picked_scores(s):
                pq = pqs[s]
                ptr = ps_trp.tile([128, 128], f32, tag="ps_tr")
                nc.tensor.transpose(ptr[0:d, 0:npick], pq[:], ident[0:npick, 0:npick])
                nc.scalar.copy(pqt[0:d, s * npick:(s + 1) * npick], ptr[0:d, 0:npick])
                last = s == n_slices - 1
                for n in range(nk // 256):
                    ps_pp = ps_ppp.tile([nused, 256], f32, tag="ps_pp")
                    nc.tensor.matmul(
                        ps_pp[s * npick:(s + 1) * npick, :],
                        lhsT=pqt[:, s * npick:(s + 1) * npick],
                        rhs=kts[s][:, n * 256:(n + 1) * 256],
                        start=True, stop=True,
                        tile_position=(0, s * npick),
                    )
                    dst = psc[s * npick:(s + 1) * npick, n * 256:(n + 1) * 256]
                    if last:
                        # DVE copy: the ACT queue is busy with the tail ladder
                        nc.vector.tensor_scalar(dst, ps_pp[s * npick:(s + 1) * npick, :],
                                                8192.0, float(1 << 19),
                                                op0=ALU.mult, op1=ALU.add)
                    else:
                        nc.scalar.activation(dst, ps_pp[s * npick:(s + 1) * npick, :],
                                             AF.Copy, bias=float(1 << 19), scale=8192.0)

            # ================= startup =================
            load_kq(0)
            load_consts()

            # ================= main pipeline =================
            for s in range(n_slices):
                alloc_lab(s)
                for t in range(nqt):
                    scores(s, t)
                    scan_count(s, t)
                    # hooks for slice s-1 work AFTER the scans in program
                    # order so they never delay the PE's scA/scB refill
                    if t == 0:
                        load_kq(s + 1)
                        if s >= 1:
                            merge(s - 1)
                    elif t == 1 and s >= 1:
                        rank_tr(s - 1)
                    elif t == 2 and s >= 1:
                        cb_build(s - 1)
                    elif t == 13 and s >= 1:
                        picked_extract(s - 1)
                    elif t == 14 and s >= 1:
                        picked_gather(s - 1)   # Pool: before the scatters
                        scatters(s - 1)
                    elif t == 15 and s >= 1:
                        picked_scores(s - 1)
                    if 3 <= t <= 10 and s >= 1:
                        rank_col(s - 1, (t - 3) * 2)
                        rank_col(s - 1, (t - 3) * 2 + 1)

            # ================= tail: slice n-1 rank + picked =================
            sl = n_slices - 1
            merge(sl)
            # tail: the ACT SEQ is backed up with copies here; the sync queue
            # is idle, so its DMA triggers fire ~2us earlier
            rank_tr(sl, q=nc.sync)
            cb_build(sl, q=nc.sync)
            ps_pk = ps_trp.tile([npick, 1], f32, tag="ps_tr")
            for t in range(nqt):
                rank_col(sl, t, tail=True)
                picked_extract_col(sl, t, ps_pk)
            picked_fin(sl, ps_pk)
            picked_gather(sl)      # Pool: ahead of the final scatters
            scatters(sl)           # drain concurrently with extraction
            picked_scores(sl)

            # ================= top-64 extraction =================
            # pack: psc_p = psc*2048 + (2047 - col); unique positive int32
            nc.vector.scalar_tensor_tensor(
                psc_p[:], psc[:], sh11[0:nused, 0:1], revi[0:nused, :],
                op0=ALU.logical_shift_left, op1=ALU.bitwise_or)
            psc_f = psc_p[:].bitcast(f32)
            # per-round idx extraction + kpi stores on non-DVE queues, so the
            # output path hides under the rounds instead of trailing them
            store_qs = [nc.sync, nc.scalar, nc.gpsimd, nc.sync]
            for r in range(sample // 8):
                pv8 = v8p.tile([nused, 8], f32, tag="pv8")
                nc.vector.max(out=pv8[:], in_=psc_f)
                nc.vector.tensor_copy(topP[:, r * 8:(r + 1) * 8], pv8[:])
                # idx = 2047 - (packed & 2047) == (packed & 2047) ^ 2047
                nc.vector.tensor_scalar(
                    topidx[:, r * 8:(r + 1) * 8],
                    topP[:, r * 8:(r + 1) * 8].bitcast(i32), 2047, 2047,
                    op0=ALU.bitwise_and, op1=ALU.bitwise_xor)
                for s in range(n_slices):
                    store_qs[s].dma_start(
                        kpi_out[s].rearrange("(j k) -> j k", k=sample)[:, r * 8:(r + 1) * 8],
                        topidx[s * npick:(s + 1) * npick, r * 8:(r + 1) * 8],
                    )
                if r < sample // 8 - 1:
                    nc.vector.match_replace(
                        out=psc_f, in_to_replace=pv8[:], in_values=psc_f,
                        imm_value=NEG_BIG,
                    )

    if split_waits:
        import concourse.mybir as mybir_mod
        _split_multi_waits(nc, mybir_mod)
    return nc


def _split_multi_waits(nc, mybir):
    """Walrus accepts only ONE sync-wait per instruction; move extras onto
    same-engine NoOps inserted before the offending instruction."""
    n = 0
    for f in nc.m.functions:
        for blk in f.blocks:
            out = []
            for inst in blk.instructions:
                si = getattr(inst, "sync_info", None)
                if si is not None and len(si.on_wait) > 1:
                    waits = list(si.on_wait)
                    for w in waits[:-1]:
                        nop = mybir.InstNoOp(
                            name=f"I-wsplit-{n}", ins=[], outs=[],
                            text_hint="wsplit",
                        )
                        n += 1
                        nop.engine = inst.engine
                        nop.sync_info = mybir.SyncInfo(on_wait=[w], on_update=[])
                        out.append(nop)
                    inst.sync_info = mybir.SyncInfo(
                        on_wait=[waits[-1]], on_update=list(si.on_update)
                    )
                out.append(inst)
            blk.instructions = out
    return nc


_BUILT = {}
LAST_RESULTS = None


def _get_nc(key=(SL, NQ, NK, D, SAMPLE)):
    if key not in _BUILT:
        _BUILT[key] = build_bass(*key)
    return _BUILT[key]


def make_iota(nqt=NQ // 128):
    # qiota[p, t] = t*128 + p
    p = np.arange(128, dtype=np.int64)[:, None]
    t = np.arange(nqt, dtype=np.int64)[None, :]
    v = (t * 128 + p)
    return v.astype(np.float32), v.astype(np.int32)


def kernel(query, key, sample_size=SAMPLE):
    from concourse.bass_utils import run_bass_kernel_spmd

    q = np.ascontiguousarray(np.asarray(query, dtype=np.float32)).reshape(B * H, NQ, D)
    k = np.ascontiguousarray(np.asarray(key, dtype=np.float32)).reshape(B * H, NK, D)
    iota_f, iota_i = make_iota()
    c64 = np.broadcast_to(
        (np.arange(NQ // SAMPLE, dtype=np.float32) * SAMPLE)[None, :], (128, NQ // SAMPLE)
    ).copy()
    revi = np.ascontiguousarray(
        np.broadcast_to((2047 - np.arange(NK, dtype=np.int32))[None, :], (128, NK))
    )

    in_maps = []
    for c in range(N_CORES):
        qs = q[c * SL:(c + 1) * SL]                       # [SL, NQ, D]
        ks = k[c * SL:(c + 1) * SL]
        in_maps.append(
            {
                "qt": np.ascontiguousarray(qs.transpose(0, 2, 1)),   # [SL, D, NQ]
                "kt": np.ascontiguousarray(ks.transpose(0, 2, 1)),
                "qrow": np.ascontiguousarray(qs.reshape(SL * NQ, D)),
                "qiota_f": iota_f,
                "qiota_i": iota_i,
                "c64": c64,
                "revi": revi,
            }
        )

    nc = _get_nc()
    trace = bool(os.environ.get("ANNS_TRACE"))
    res = run_bass_kernel_spmd(
        nc, in_maps, core_ids=list(range(N_CORES)), trace=trace
    )
    global LAST_RESULTS
    LAST_RESULTS = res
    qsi = np.concatenate(
        [res.results[i]["qsi"].reshape(SL, NQ) for i in range(N_CORES)], axis=0
    ).reshape(B, H, NQ)
    kpi = np.concatenate(
        [res.results[i]["kpi"].reshape(SL, NQ) for i in range(N_CORES)], axis=0
    ).reshape(B, H, NQ)
    return qsi.astype(np.int32), kpi.astype(np.int32)


if __name__ == "__main__":
    rng = np.random.default_rng(0)
    q = rng.normal(size=(B, H, NQ, D)).astype(np.float32)
    k = rng.normal(size=(B, H, NK, D)).astype(np.float32)
    out = kernel(q, k, SAMPLE)
    print([o.shape for o in out])
